# revision 7
# baseline (speedup 1.0000x reference)
"""PVT-style spatial-reduction attention on 8 TRN2 NeuronCores (Bass/Tile).

Strategy: data-parallel over batch (16 images -> 2 per core). Each core runs an
identical single-core program on its shard; no collectives.

Host-side prep (inside kernel(), part of sharding/layout):
  - x transposed to channel-major xT [2, 256, 4096] so matmul operands need no
    on-device transposition of the big activation.
  - attention scale hd^-0.5 folded into Wq; LayerNorm gamma/beta folded into
    Wkv algebraically (exact); conv weights pre-transposed per tap.

Device pipeline per batch (all matmuls float32r):
  qT = Wq^T @ xT                       (feature-major q)
  xr = sum over 16 conv taps of gathered-xT^T @ w_tap   (strided-gather lhsT)
  LN over free dim (quake rsqrt on DVE), PE-transpose of x_norm (small)
  kT = Wk^T @ xnT ; v = xn @ Wv
  per 512-row block, per head:
    S^T = kT_h^T @ qT_h  (keys on partitions)  -> exp on ScalarE (PSUM->SBUF)
    sums = ones32^T @ P^T  (32-replicated row sums via matmul)
    O^T  = v_h^T @ P^T     (unnormalized)
    R = reciprocal_approx_fast(sums); O-norm fused into PSUM->SBUF move
  out = O_norm^T^T @ Wp (+bp via K=1 matmul)  -> natural layout -> DMA out
"""

import os
import sys
from contextlib import ExitStack

if "/opt/trn_rl_repo" not in sys.path:
    sys.path.insert(0, "/opt/trn_rl_repo")

import numpy as np
import ml_dtypes

import concourse.bass as bass
import concourse.bacc as bacc
import concourse.tile as tile
from concourse import mybir
from concourse.bass_utils import run_bass_kernel_spmd

N_CORES = 8
B, N, C = 16, 4096, 256
B_LOC = B // N_CORES
H8, HD, M = 8, 32, 256
NBLK, BLK = 8, 512
F32 = mybir.dt.float32
F32R = mybir.dt.float32r
BF16 = mybir.dt.bfloat16
I32 = mybir.dt.int32
AF = mybir.ActivationFunctionType
OP = mybir.AluOpType
AX = mybir.AxisListType

KERNEL_STATS = {}


def _r(ap):
    return ap


def _kernel_body(ctx, tc, out, ins, with_bp):
    nc = tc.nc
    (xT_d, wq_d, wk_d, wv_d, srw_d, srb_d, bk_d, bv_d, wp_d, bp_d,
     eye_d, ones32_d, onesr_d) = ins

    consts = ctx.enter_context(tc.tile_pool(name="consts", bufs=1))
    sb_xT = ctx.enter_context(tc.tile_pool(name="sb_xT", bufs=2))
    sb_qT = ctx.enter_context(tc.tile_pool(name="sb_qT", bufs=1))
    sb_oT = ctx.enter_context(tc.tile_pool(name="sb_oT", bufs=1))
    sb_pT = ctx.enter_context(tc.tile_pool(name="sb_pT", bufs=8))
    sb_srw = ctx.enter_context(tc.tile_pool(name="sb_srw", bufs=8))
    sb_kv = ctx.enter_context(tc.tile_pool(name="sb_kv", bufs=2))
    sb_ln = ctx.enter_context(tc.tile_pool(name="sb_ln", bufs=2))
    sb_R = ctx.enter_context(tc.tile_pool(name="sb_R", bufs=2))
    sb_st = ctx.enter_context(tc.tile_pool(name="sb_st", bufs=3))
    ps_s = ctx.enter_context(tc.tile_pool(name="ps_s", bufs=2, space="PSUM"))
    ps_att = ctx.enter_context(tc.tile_pool(name="ps_att", bufs=2, space="PSUM"))
    ps_gen = ctx.enter_context(tc.tile_pool(name="ps_gen", bufs=2, space="PSUM"))

    # ---- constants / weights (resident) ----
    def cload(name, src, shape, dtype=F32):
        t = consts.tile(shape, dtype, tag=name, name=name)
        nc.sync.dma_start(t[:], src)
        return t

    wq_sb = [cload(f"wq{k}", wq_d[128 * k:128 * (k + 1), :], [128, C], F32R) for k in range(2)]
    wk_sb = [cload(f"wk{k}", wk_d[128 * k:128 * (k + 1), :], [128, C], F32R) for k in range(2)]
    wv_sb = [cload(f"wv{k}", wv_d[128 * k:128 * (k + 1), :], [128, C], F32R) for k in range(2)]
    wp_sb = [cload(f"wp{k}", wp_d[128 * k:128 * (k + 1), :], [128, C], F32R) for k in range(2)]
    srb_sb = cload("srb", srb_d[:, :], [128, C])
    bv_sb = cload("bv", bv_d[:, :], [128, C])
    bk_sb = [cload(f"bk{k}", bk_d[k], [128, 1]) for k in range(2)]
    eye_sb = cload("eye", eye_d[:, :], [128, 128])
    ones32_sb = cload("ones32", ones32_d[:, :], [128, 32], BF16)
    onesr_sb = cload("onesr", onesr_d[:, :], [1, 128], F32R)
    bp_sb = cload("bp", bp_d[:, :], [1, C], F32R)

    magic_t = consts.tile([128, 1], I32, tag="magic", name="magic")
    nc.gpsimd.memset(magic_t[:], 0x5F3759DF)
    c15_t = consts.tile([128, 1], F32, tag="c15", name="c15")
    nc.gpsimd.memset(c15_t[:], 1.5)

    for b in range(B_LOC):
        # ---- load xT ----
        xT_sb = []
        for k in range(2):
            xt = sb_xT.tile([128, N], F32R, tag="xT", name=f"xt{b}{k}")
            nc.sync.dma_start(xt[:], xT_d[b, 128 * k:128 * (k + 1), :])
            xT_sb.append(xt)

        # ---- conv (16 taps, accumulate in PSUM) ----
        # x arrives im2col-permuted: free index tap*256 + m, m = out position.
        ps_conv = [ps_gen.tile([128, C], F32, tag="g", name=f"psc{b}{mo}")
                   for mo in range(2)]
        for tap in range(16):
            srw_t = sb_srw.tile([128, 2 * C], F32R, tag="srw", name=f"srw{b}{tap}")
            nc.sync.dma_start(
                srw_t.rearrange("p (k o) -> p k o", k=2),
                srw_d[tap].rearrange("(k i) o -> i k o", k=2))
            for mo in range(2):
                for ki in range(2):
                    lhsT = xT_sb[ki][:, 256 * tap + 128 * mo:
                                     256 * tap + 128 * (mo + 1)]
                    nc.tensor.matmul(
                        ps_conv[mo][:],
                        _r(lhsT),
                        _r(srw_t[:, C * ki:C * (ki + 1)]),
                        start=(tap == 0 and ki == 0),
                        stop=(tap == 15 and ki == 1),
                    )

        # ---- LayerNorm (free-dim stats; gamma/beta folded into Wkv) ----
        xn_sb = []
        for mo in range(2):
            xb = sb_ln.tile([128, C], F32, tag="xb", name=f"xb{b}{mo}")
            nc.vector.tensor_add(xb[:], ps_conv[mo][:], srb_sb[:])
            ssum = sb_ln.tile([128, 1], F32, tag="ssum", name=f"ssum{b}{mo}")
            nc.vector.tensor_reduce(ssum[:], xb[:], axis=AX.X, op=OP.add)
            mu = sb_ln.tile([128, 1], F32, tag="mu", name=f"mu{b}{mo}")
            nc.vector.tensor_scalar_mul(mu[:], ssum[:], 1.0 / C)
            xc = sb_ln.tile([128, C], F32, tag="xc", name=f"xc{b}{mo}")
            nc.vector.tensor_scalar_sub(xc[:], xb[:], mu[:, 0:1])
            sq = sb_ln.tile([128, C], F32, tag="sq", name=f"sq{b}{mo}")
            vraw = sb_ln.tile([128, 1], F32, tag="vraw", name=f"vraw{b}{mo}")
            nc.vector.scalar_tensor_tensor(
                sq[:], xc[:], 0.0, xc[:], op0=OP.add, op1=OP.mult,
                accum_out=vraw[:, 0:1])
            veps = sb_ln.tile([128, 1], F32, tag="veps", name=f"veps{b}{mo}")
            nc.vector.tensor_scalar(veps[:], vraw[:], 1.0 / C, 1e-5,
                                    op0=OP.mult, op1=OP.add)
            vh = sb_ln.tile([128, 1], F32, tag="vh", name=f"vh{b}{mo}")
            nc.vector.tensor_scalar_mul(vh[:], veps[:], -0.5)
            # quake rsqrt seed + 3 Newton iterations
            sh = sb_ln.tile([128, 1], I32, tag="sh", name=f"sh{b}{mo}")
            nc.vector.tensor_scalar(sh[:], veps[:].bitcast(I32), 1, None,
                                    op0=OP.logical_shift_right)
            y = sb_ln.tile([128, 1], F32, tag="y", name=f"y{b}{mo}")
            nc.vector.scalar_tensor_tensor(
                y[:].bitcast(I32), magic_t[:], 0, sh[:],
                op0=OP.bypass, op1=OP.subtract)
            for it in range(3):
                yy = sb_ln.tile([128, 1], F32, tag=f"yy{it}", name=f"yy{b}{mo}{it}")
                nc.vector.tensor_mul(yy[:], y[:], y[:])
                t2 = sb_ln.tile([128, 1], F32, tag=f"t2{it}", name=f"t2{b}{mo}{it}")
                nc.vector.scalar_tensor_tensor(
                    t2[:], yy[:], vh[:, 0:1], c15_t[:],
                    op0=OP.mult, op1=OP.add)
                y2 = sb_ln.tile([128, 1], F32, tag=f"y2{it}", name=f"ynew{b}{mo}{it}")
                nc.vector.tensor_mul(y2[:], y[:], t2[:])
                y = y2
            xn = sb_ln.tile([128, C], F32, tag="xn", name=f"xn{b}{mo}")
            nc.vector.tensor_scalar_mul(xn[:], xc[:], y[:, 0:1])
            xn_sb.append(xn)

        # ---- transpose x_norm -> xnT [ci, pos] ----
        xnT_sb = []
        for i in range(2):
            xnT = sb_kv.tile([128, M], F32R, tag=f"xnT{i}", name=f"xnT{b}{i}")
            xnT_sb.append(xnT)
        for i in range(2):
            for j in range(2):
                ps_t = ps_gen.tile([128, 128], F32, tag="g", name=f"pst{b}{i}{j}")
                nc.tensor.transpose(ps_t[:], xn_sb[j][:, 128 * i:128 * (i + 1)],
                                    eye_sb[:])
                nc.vector.tensor_copy(xnT_sb[i][:, 128 * j:128 * (j + 1)], ps_t[:])

        # ---- kT = Wk^T @ xnT (+bias_k), v = xn @ Wv (+bias_v) ----
        kT_sb, v_sb = [], []
        for mo in range(2):
            ps_k = ps_gen.tile([128, M], F32, tag="g", name=f"psk{b}{mo}")
            for ki in range(2):
                nc.tensor.matmul(ps_k[:], _r(wk_sb[ki][:, 128 * mo:128 * (mo + 1)]),
                                 _r(xnT_sb[ki][:]),
                                 start=(ki == 0), stop=(ki == 1))
            kT = sb_kv.tile([128, M], F32R, tag=f"kT{mo}", name=f"kT{b}{mo}")
            nc.vector.tensor_scalar_add(kT[:], ps_k[:], bk_sb[mo][:, 0:1])
            kT_sb.append(kT)
        for mo in range(2):
            ps_v = ps_gen.tile([128, C], F32, tag="g", name=f"psv{b}{mo}")
            for ki in range(2):
                nc.tensor.matmul(ps_v[:], _r(xnT_sb[ki][:, 128 * mo:128 * (mo + 1)]),
                                 _r(wv_sb[ki][:]),
                                 start=(ki == 0), stop=(ki == 1))
            v = sb_kv.tile([128, C], BF16, tag=f"v{mo}", name=f"v{b}{mo}")
            nc.vector.tensor_add(v[:], ps_v[:], bv_sb[:])
            v_sb.append(v)

        # ---- qT = Wq^T @ xT ----
        qT_sb = [sb_qT.tile([128, N], F32R, tag=f"qT{k}", name=f"qT{b}{k}")
                 for k in range(2)]
        for blk in range(NBLK):
            for mo in range(2):
                ps_q = ps_gen.tile([128, BLK], F32, tag="g", name=f"psq{b}{blk}{mo}")
                for ki in range(2):
                    nc.tensor.matmul(
                        ps_q[:], _r(wq_sb[ki][:, 128 * mo:128 * (mo + 1)]),
                        _r(xT_sb[ki][:, BLK * blk:BLK * (blk + 1)]),
                        start=(ki == 0), stop=(ki == 1))
                nc.vector.tensor_copy(qT_sb[mo][:, BLK * blk:BLK * (blk + 1)],
                                      ps_q[:])

        # ---- attention blocks ----
        oT_sb = [sb_oT.tile([128, N], F32R, tag=f"oT{k}", name=f"oT{b}{k}")
                 for k in range(2)]
        for blk in range(NBLK):
            for sg in range(2):
                ps_sum = ps_att.tile([128, BLK], F32, tag="att",
                                     name=f"pssum{b}{blk}{sg}")
                ps_o = ps_att.tile([128, BLK], F32, tag="att",
                                   name=f"pso{b}{blk}{sg}")
                for hl in range(4):
                    hh = 4 * sg + hl
                    st_t = ps_s.tile([128, 2 * BLK], F32, tag="s",
                                     name=f"psst{b}{blk}{hh}")
                    for ko in range(2):
                        nc.tensor.matmul(
                            st_t[:, BLK * ko:BLK * (ko + 1)],
                            _r(kT_sb[sg][32 * hl:32 * hl + 32,
                                         128 * ko:128 * (ko + 1)]),
                            _r(qT_sb[sg][32 * hl:32 * hl + 32,
                                         BLK * blk:BLK * (blk + 1)]),
                            start=True, stop=True,
                            tile_position=(32 * hl, 0),
                        )
                    pt = sb_pT.tile([128, 2 * BLK], BF16, tag="pT",
                                    name=f"pt{b}{blk}{hh}")
                    nc.scalar.activation(pt[:], st_t[:], AF.Exp)
                    for ko in range(2):
                        nc.tensor.matmul(
                            ps_sum[32 * hl:32 * hl + 32, :],
                            _r(ones32_sb[:]),
                            _r(pt[:, BLK * ko:BLK * (ko + 1)]),
                            start=(ko == 0), stop=(ko == 1),
                            tile_position=(0, 32 * hl),
                            skip_group_check=True,
                        )
                    for ko in range(2):
                        nc.tensor.matmul(
                            ps_o[32 * hl:32 * hl + 32, :],
                            _r(v_sb[ko][:, 32 * hh:32 * hh + 32]),
                            _r(pt[:, BLK * ko:BLK * (ko + 1)]),
                            start=(ko == 0), stop=(ko == 1),
                            tile_position=(0, 32 * hl),
                            skip_group_check=True,
                        )
                R_t = sb_R.tile([128, BLK], F32, tag="R", name=f"R{b}{blk}{sg}")
                nc.vector.reciprocal_approx_fast(R_t[:], ps_sum[:])
                nc.vector.tensor_mul(oT_sb[sg][:, BLK * blk:BLK * (blk + 1)],
                                     ps_o[:], R_t[:])

            # ---- proj for this block's 4 row-chunks ----
            for rbp in range(2):
                ps_pj = ps_gen.tile([128, BLK], F32, tag="g",
                                    name=f"pspj{b}{blk}{rbp}")
                for half in range(2):
                    rb = 4 * blk + 2 * rbp + half
                    for ki in range(2):
                        nc.tensor.matmul(
                            ps_pj[:, C * half:C * (half + 1)],
                            _r(oT_sb[ki][:, 128 * rb:128 * (rb + 1)]),
                            _r(wp_sb[ki][:]),
                            start=(ki == 0),
                            stop=(ki == 1 and not with_bp))
                    if with_bp:
                        nc.tensor.matmul(
                            ps_pj[:, C * half:C * (half + 1)],
                            _r(onesr_sb[0:1, :]),
                            _r(bp_sb[0:1, :]),
                            start=False, stop=True)
                st = sb_st.tile([128, BLK], F32, tag="st", name=f"st{b}{blk}{rbp}")
                nc.vector.tensor_copy(st[:], ps_pj[:])
                r0 = 128 * (4 * blk + 2 * rbp)
                dst = out[b, r0:r0 + 256, :].rearrange("(p r) c -> r p c", p=2)
                nc.sync.dma_start(dst, st.rearrange("r (p c) -> r p c", p=2))


def build(with_bp):
    nc = bacc.Bacc("TRN2", target_bir_lowering=False, debug=False,
                   enable_asserts=True)

    def din(name, shape, dtype=F32):
        return nc.dram_tensor(name, shape, dtype, kind="ExternalInput").ap()

    ins = [
        din("xT", [B_LOC, C, N], F32R),
        din("wq", [C, C], F32R),
        din("wk", [C, C], F32R),
        din("wv", [C, C], F32R),
        din("srw", [16, C, C], F32R),
        din("srb", [128, C]),
        din("bk", [2, 128, 1]),
        din("bv", [128, C]),
        din("wp", [C, C], F32R),
        din("bp", [1, C], F32R),
        din("eye", [128, 128]),
        din("ones32", [128, 32], BF16),
        din("onesr", [1, 128], F32R),
    ]
    out = nc.dram_tensor("out", [B_LOC, N, C], F32, kind="ExternalOutput").ap()

    with tile.TileContext(nc) as tc:
        with ExitStack() as ctx:
            _kernel_body(ctx, tc, out, ins, with_bp)
    nc.compile()
    return nc


def host_prep(inputs):
    """Shared (non-x) host-side tensors, from the full input dict."""
    Wq = np.asarray(inputs["Wq"], np.float32)
    Wkv = np.asarray(inputs["Wkv"], np.float32)
    sr_w = np.asarray(inputs["sr_w"], np.float32)
    sr_b = np.asarray(inputs["sr_b"], np.float32)
    ln_g = np.asarray(inputs["ln_g"], np.float32)
    ln_b = np.asarray(inputs["ln_b"], np.float32)
    Wp = np.asarray(inputs["Wp"], np.float32)
    bp = np.asarray(inputs["bp"], np.float32)

    wq = (Wq * (HD ** -0.5)).astype(np.float32)
    wk = (ln_g[:, None] * Wkv[:, :C]).astype(np.float32)
    wv = (ln_g[:, None] * Wkv[:, C:]).astype(np.float32)
    bias_kv = (ln_b @ Wkv).astype(np.float32)
    srwT = np.ascontiguousarray(
        sr_w.transpose(2, 3, 1, 0).reshape(16, C, C))

    shared = {
        "wq": wq,
        "wk": wk,
        "wv": wv,
        "srw": srwT,
        "srb": np.ascontiguousarray(np.broadcast_to(sr_b, (128, C))),
        "bk": np.ascontiguousarray(bias_kv[:C].reshape(2, 128, 1)),
        "bv": np.ascontiguousarray(np.broadcast_to(bias_kv[C:], (128, C))),
        "wp": Wp,
        "bp": np.ascontiguousarray(bp.reshape(1, C)),
        "eye": np.eye(128, dtype=np.float32),
        "ones32": np.ones((128, 32), ml_dtypes.bfloat16),
        "onesr": np.ones((1, 128), np.float32),
    }
    with_bp = bool(np.any(bp != 0))
    return shared, with_bp


_NC_CACHE = {}


def get_nc(with_bp):
    if with_bp not in _NC_CACHE:
        _NC_CACHE[with_bp] = build(with_bp)
    return _NC_CACHE[with_bp]


def _im2col_perm():
    """idx[tap*256 + m] = spatial row index n for the stride-4 4x4 conv."""
    tap = np.arange(16)
    kh, kw = tap // 4, tap % 4
    m = np.arange(256)
    R, Cc = m // 16, m % 16
    idx = (256 * R[None, :] + 4 * Cc[None, :]
           + 64 * kh[:, None] + kw[:, None])
    return idx.reshape(-1)


IM2COL_IDX = _im2col_perm()


def make_in_maps(inputs):
    x = np.asarray(inputs["x"], np.float32)
    shared, with_bp = host_prep(inputs)
    in_maps = []
    for c in range(N_CORES):
        xc = x[B_LOC * c:B_LOC * (c + 1)]
        xT = np.ascontiguousarray(xc.transpose(0, 2, 1)[:, :, IM2COL_IDX])
        m = dict(shared)
        m["xT"] = xT
        in_maps.append(m)
    return in_maps, with_bp


def kernel(**inputs):
    in_maps, with_bp = make_in_maps(inputs)
    nc = get_nc(with_bp)
    res = run_bass_kernel_spmd(nc, in_maps, core_ids=list(range(N_CORES)))
    KERNEL_STATS["exec_time_ns"] = res.exec_time_ns
    KERNEL_STATS["mean_exec_time_ns"] = res.mean_exec_time_ns
    KERNEL_STATS["trace"] = res.instructions_and_trace
    out_perm = np.concatenate(
        [res.results[c]["out"] for c in range(N_CORES)], axis=0)
    out = np.empty_like(out_perm)
    out[:, IM2COL_IDX, :] = out_perm
    return out


# revision 8
# speedup vs baseline: 1.2685x; 1.2685x over previous
"""PVT-style spatial-reduction attention on 8 TRN2 NeuronCores (Bass/Tile).

Strategy: data-parallel over batch (16 images -> 2 per core). Each core runs an
identical single-core program on its shard; no collectives.

Host-side prep (inside kernel(), part of sharding/layout):
  - x transposed to channel-major xT [2, 256, 4096] so matmul operands need no
    on-device transposition of the big activation.
  - attention scale hd^-0.5 folded into Wq; LayerNorm gamma/beta folded into
    Wkv algebraically (exact); conv weights pre-transposed per tap.

Device pipeline per batch (all matmuls float32r):
  qT = Wq^T @ xT                       (feature-major q)
  xr = sum over 16 conv taps of gathered-xT^T @ w_tap   (strided-gather lhsT)
  LN over free dim (quake rsqrt on DVE), PE-transpose of x_norm (small)
  kT = Wk^T @ xnT ; v = xn @ Wv
  per 512-row block, per head:
    S^T = kT_h^T @ qT_h  (keys on partitions)  -> exp on ScalarE (PSUM->SBUF)
    sums = ones32^T @ P^T  (32-replicated row sums via matmul)
    O^T  = v_h^T @ P^T     (unnormalized)
    R = reciprocal_approx_fast(sums); O-norm fused into PSUM->SBUF move
  out = O_norm^T^T @ Wp (+bp via K=1 matmul)  -> natural layout -> DMA out
"""

import os
import sys
from contextlib import ExitStack

if "/opt/trn_rl_repo" not in sys.path:
    sys.path.insert(0, "/opt/trn_rl_repo")

import numpy as np
import ml_dtypes

import concourse.bass as bass
import concourse.bacc as bacc
import concourse.tile as tile
from concourse import mybir
from concourse.bass_utils import run_bass_kernel_spmd

N_CORES = 8
B, N, C = 16, 4096, 256
B_LOC = B // N_CORES
H8, HD, M = 8, 32, 256
NBLK, BLK = 8, 512
F32 = mybir.dt.float32
F32R = mybir.dt.float32r
BF16 = mybir.dt.bfloat16
I32 = mybir.dt.int32
AF = mybir.ActivationFunctionType
OP = mybir.AluOpType
AX = mybir.AxisListType

KERNEL_STATS = {}


def _r(ap):
    return ap


def _kernel_body(ctx, tc, out, ins, with_bp):
    nc = tc.nc
    (xT_d, wq_d, wk_d, wv_d, srw_d, srb_d, bk_d, bv_d, wp_d, bp_d,
     eye_d, ones32_d, onesr_d) = ins

    consts = ctx.enter_context(tc.tile_pool(name="consts", bufs=1))
    sb_xT = ctx.enter_context(tc.tile_pool(name="sb_xT", bufs=2))
    sb_qT = ctx.enter_context(tc.tile_pool(name="sb_qT", bufs=1))
    sb_oT = ctx.enter_context(tc.tile_pool(name="sb_oT", bufs=1))
    sb_pT = ctx.enter_context(tc.tile_pool(name="sb_pT", bufs=8))
    sb_srw = ctx.enter_context(tc.tile_pool(name="sb_srw", bufs=8))
    sb_kv = ctx.enter_context(tc.tile_pool(name="sb_kv", bufs=2))
    sb_ln = ctx.enter_context(tc.tile_pool(name="sb_ln", bufs=2))
    sb_R = ctx.enter_context(tc.tile_pool(name="sb_R", bufs=2))
    sb_st = ctx.enter_context(tc.tile_pool(name="sb_st", bufs=3))
    ps_s = ctx.enter_context(tc.tile_pool(name="ps_s", bufs=2, space="PSUM"))
    ps_att = ctx.enter_context(tc.tile_pool(name="ps_att", bufs=2, space="PSUM"))
    ps_gen = ctx.enter_context(tc.tile_pool(name="ps_gen", bufs=2, space="PSUM"))

    # ---- constants / weights (resident) ----
    def cload(name, src, shape, dtype=F32):
        t = consts.tile(shape, dtype, tag=name, name=name)
        nc.sync.dma_start(t[:], src)
        return t

    wq_sb = [cload(f"wq{k}", wq_d[128 * k:128 * (k + 1), :], [128, C], F32R) for k in range(2)]
    wk_sb = [cload(f"wk{k}", wk_d[128 * k:128 * (k + 1), :], [128, C], F32R) for k in range(2)]
    wv_sb = [cload(f"wv{k}", wv_d[128 * k:128 * (k + 1), :], [128, C], F32R) for k in range(2)]
    wp_sb = [cload(f"wp{k}", wp_d[128 * k:128 * (k + 1), :], [128, C], F32R) for k in range(2)]
    srb_sb = cload("srb", srb_d[:, :], [128, C])
    bv_sb = cload("bv", bv_d[:, :], [128, C])
    bk_sb = [cload(f"bk{k}", bk_d[k], [128, 1]) for k in range(2)]
    eye_sb = cload("eye", eye_d[:, :], [128, 128])
    ones32_sb = cload("ones32", ones32_d[:, :], [128, 32], BF16)
    onesr_sb = cload("onesr", onesr_d[:, :], [1, 128], F32R)
    bp_sb = cload("bp", bp_d[:, :], [1, C], F32R)

    magic_t = consts.tile([128, 1], I32, tag="magic", name="magic")
    nc.gpsimd.memset(magic_t[:], 0x5F3759DF)
    c15_t = consts.tile([128, 1], F32, tag="c15", name="c15")
    nc.gpsimd.memset(c15_t[:], 1.5)

    for b in range(B_LOC):
        # ---- load xT ----
        xT_sb = []
        for k in range(2):
            xt = sb_xT.tile([128, N], F32R, tag="xT", name=f"xt{b}{k}")
            for q4 in range(4):
                nc.sync.dma_start(xt[:, 1024 * q4:1024 * (q4 + 1)],
                                  xT_d[b, 128 * k:128 * (k + 1),
                                       1024 * q4:1024 * (q4 + 1)])
            xT_sb.append(xt)

        # ---- conv (16 taps, accumulate in PSUM) ----
        # x arrives im2col-permuted: free index tap*256 + m, m = out position.
        ps_conv = [ps_gen.tile([128, C], F32, tag="g", name=f"psc{b}{mo}")
                   for mo in range(2)]
        for tap in range(16):
            srw_t = sb_srw.tile([128, 2 * C], F32R, tag="srw", name=f"srw{b}{tap}")
            nc.sync.dma_start(
                srw_t.rearrange("p (k o) -> p k o", k=2),
                srw_d[tap].rearrange("(k i) o -> i k o", k=2))
            for mo in range(2):
                for ki in range(2):
                    lhsT = xT_sb[ki][:, 256 * tap + 128 * mo:
                                     256 * tap + 128 * (mo + 1)]
                    nc.tensor.matmul(
                        ps_conv[mo][:],
                        _r(lhsT),
                        _r(srw_t[:, C * ki:C * (ki + 1)]),
                        start=(tap == 0 and ki == 0),
                        stop=(tap == 15 and ki == 1),
                    )

        # ---- LayerNorm (free-dim stats; gamma/beta folded into Wkv) ----
        xn_sb = []
        for mo in range(2):
            xb = sb_ln.tile([128, C], F32, tag="xb", name=f"xb{b}{mo}")
            nc.vector.tensor_add(xb[:], ps_conv[mo][:], srb_sb[:])
            ssum = sb_ln.tile([128, 1], F32, tag="ssum", name=f"ssum{b}{mo}")
            nc.vector.tensor_reduce(ssum[:], xb[:], axis=AX.X, op=OP.add)
            mu = sb_ln.tile([128, 1], F32, tag="mu", name=f"mu{b}{mo}")
            nc.vector.tensor_scalar_mul(mu[:], ssum[:], 1.0 / C)
            xc = sb_ln.tile([128, C], F32, tag="xc", name=f"xc{b}{mo}")
            nc.vector.tensor_scalar_sub(xc[:], xb[:], mu[:, 0:1])
            sq = sb_ln.tile([128, C], F32, tag="sq", name=f"sq{b}{mo}")
            vraw = sb_ln.tile([128, 1], F32, tag="vraw", name=f"vraw{b}{mo}")
            nc.vector.scalar_tensor_tensor(
                sq[:], xc[:], 0.0, xc[:], op0=OP.add, op1=OP.mult,
                accum_out=vraw[:, 0:1])
            veps = sb_ln.tile([128, 1], F32, tag="veps", name=f"veps{b}{mo}")
            nc.vector.tensor_scalar(veps[:], vraw[:], 1.0 / C, 1e-5,
                                    op0=OP.mult, op1=OP.add)
            vh = sb_ln.tile([128, 1], F32, tag="vh", name=f"vh{b}{mo}")
            nc.vector.tensor_scalar_mul(vh[:], veps[:], -0.5)
            # quake rsqrt seed + 3 Newton iterations
            sh = sb_ln.tile([128, 1], I32, tag="sh", name=f"sh{b}{mo}")
            nc.vector.tensor_scalar(sh[:], veps[:].bitcast(I32), 1, None,
                                    op0=OP.logical_shift_right)
            y = sb_ln.tile([128, 1], F32, tag="y", name=f"y{b}{mo}")
            nc.vector.scalar_tensor_tensor(
                y[:].bitcast(I32), magic_t[:], 0, sh[:],
                op0=OP.bypass, op1=OP.subtract)
            for it in range(3):
                yy = sb_ln.tile([128, 1], F32, tag=f"yy{it}", name=f"yy{b}{mo}{it}")
                nc.vector.tensor_mul(yy[:], y[:], y[:])
                t2 = sb_ln.tile([128, 1], F32, tag=f"t2{it}", name=f"t2{b}{mo}{it}")
                nc.vector.scalar_tensor_tensor(
                    t2[:], yy[:], vh[:, 0:1], c15_t[:],
                    op0=OP.mult, op1=OP.add)
                y2 = sb_ln.tile([128, 1], F32, tag=f"y2{it}", name=f"ynew{b}{mo}{it}")
                nc.vector.tensor_mul(y2[:], y[:], t2[:])
                y = y2
            xn = sb_ln.tile([128, C], F32, tag="xn", name=f"xn{b}{mo}")
            nc.vector.tensor_scalar_mul(xn[:], xc[:], y[:, 0:1])
            xn_sb.append(xn)

        # ---- transpose x_norm -> xnT [ci, pos] ----
        xnT_sb = []
        for i in range(2):
            xnT = sb_kv.tile([128, M], F32R, tag=f"xnT{i}", name=f"xnT{b}{i}")
            xnT_sb.append(xnT)
        for i in range(2):
            for j in range(2):
                ps_t = ps_gen.tile([128, 128], F32, tag="g", name=f"pst{b}{i}{j}")
                nc.tensor.transpose(ps_t[:], xn_sb[j][:, 128 * i:128 * (i + 1)],
                                    eye_sb[:])
                nc.vector.tensor_copy(xnT_sb[i][:, 128 * j:128 * (j + 1)], ps_t[:])

        # ---- kT = Wk^T @ xnT (+bias_k), v = xn @ Wv (+bias_v) ----
        kT_sb, v_sb = [], []
        for mo in range(2):
            ps_k = ps_gen.tile([128, M], F32, tag="g", name=f"psk{b}{mo}")
            for ki in range(2):
                nc.tensor.matmul(ps_k[:], _r(wk_sb[ki][:, 128 * mo:128 * (mo + 1)]),
                                 _r(xnT_sb[ki][:]),
                                 start=(ki == 0), stop=(ki == 1))
            kT = sb_kv.tile([128, M], F32R, tag=f"kT{mo}", name=f"kT{b}{mo}")
            nc.vector.tensor_scalar_add(kT[:], ps_k[:], bk_sb[mo][:, 0:1])
            kT_sb.append(kT)
        for mo in range(2):
            ps_v = ps_gen.tile([128, C], F32, tag="g", name=f"psv{b}{mo}")
            for ki in range(2):
                nc.tensor.matmul(ps_v[:], _r(xnT_sb[ki][:, 128 * mo:128 * (mo + 1)]),
                                 _r(wv_sb[ki][:]),
                                 start=(ki == 0), stop=(ki == 1))
            v = sb_kv.tile([128, C], BF16, tag=f"v{mo}", name=f"v{b}{mo}")
            nc.vector.tensor_add(v[:], ps_v[:], bv_sb[:])
            v_sb.append(v)

        # ---- qT = Wq^T @ xT ----
        qT_sb = [sb_qT.tile([128, N], F32R, tag=f"qT{k}", name=f"qT{b}{k}")
                 for k in range(2)]
        for blk in range(NBLK):
            for mo in range(2):
                ps_q = ps_gen.tile([128, BLK], F32, tag="g", name=f"psq{b}{blk}{mo}")
                for ki in range(2):
                    nc.tensor.matmul(
                        ps_q[:], _r(wq_sb[ki][:, 128 * mo:128 * (mo + 1)]),
                        _r(xT_sb[ki][:, BLK * blk:BLK * (blk + 1)]),
                        start=(ki == 0), stop=(ki == 1))
                nc.vector.tensor_copy(qT_sb[mo][:, BLK * blk:BLK * (blk + 1)],
                                      ps_q[:])

        # ---- attention blocks ----
        oT_sb = [sb_oT.tile([128, N], F32R, tag=f"oT{k}", name=f"oT{b}{k}")
                 for k in range(2)]
        for blk in range(NBLK):
            for sg in range(2):
                # S^T + exp, two heads at a time (adjacent row strips overlap
                # on the PE array); then 4-way col-packed sums and O matmuls.
                pts = []
                for pr in range(2):
                    st_ts = []
                    for hp in range(2):
                        hl = 2 * pr + hp
                        hh = 4 * sg + hl
                        st_t = ps_s.tile([128, 2 * BLK], F32, tag="s",
                                         name=f"psst{b}{blk}{hh}")
                        st_ts.append(st_t)
                    for ko in range(2):
                        for hp in range(2):
                            hl = 2 * pr + hp
                            nc.tensor.matmul(
                                st_ts[hp][:, BLK * ko:BLK * (ko + 1)],
                                _r(kT_sb[sg][32 * hl:32 * hl + 32,
                                             128 * ko:128 * (ko + 1)]),
                                _r(qT_sb[sg][32 * hl:32 * hl + 32,
                                             BLK * blk:BLK * (blk + 1)]),
                                start=True, stop=True,
                                tile_position=(32 * hl, 0),
                            )
                    for hp in range(2):
                        hh = 4 * sg + 2 * pr + hp
                        pt = sb_pT.tile([128, 2 * BLK], BF16, tag="pT",
                                        name=f"pt{b}{blk}{hh}")
                        nc.scalar.activation(pt[:], st_ts[hp][:], AF.Exp)
                        pts.append(pt)
                ps_sum = ps_att.tile([128, BLK], F32, tag="att",
                                     name=f"pssum{b}{blk}{sg}")
                ps_o = ps_att.tile([128, BLK], F32, tag="att",
                                   name=f"pso{b}{blk}{sg}")
                for ko in range(2):
                    for hl in range(4):
                        nc.tensor.matmul(
                            ps_sum[32 * hl:32 * hl + 32, :],
                            _r(ones32_sb[:]),
                            _r(pts[hl][:, BLK * ko:BLK * (ko + 1)]),
                            start=(ko == 0), stop=(ko == 1),
                            tile_position=(0, 32 * hl),
                            skip_group_check=True,
                        )
                for ko in range(2):
                    for hl in range(4):
                        hh = 4 * sg + hl
                        nc.tensor.matmul(
                            ps_o[32 * hl:32 * hl + 32, :],
                            _r(v_sb[ko][:, 32 * hh:32 * hh + 32]),
                            _r(pts[hl][:, BLK * ko:BLK * (ko + 1)]),
                            start=(ko == 0), stop=(ko == 1),
                            tile_position=(0, 32 * hl),
                            skip_group_check=True,
                        )
                R_t = sb_R.tile([128, BLK], F32, tag="R", name=f"R{b}{blk}{sg}")
                nc.vector.reciprocal_approx_fast(R_t[:], ps_sum[:])
                nc.vector.tensor_mul(oT_sb[sg][:, BLK * blk:BLK * (blk + 1)],
                                     ps_o[:], R_t[:])

            # ---- proj for this block's 4 row-chunks ----
            for rbp in range(2):
                ps_pj = ps_gen.tile([128, BLK], F32, tag="g",
                                    name=f"pspj{b}{blk}{rbp}")
                for half in range(2):
                    rb = 4 * blk + 2 * rbp + half
                    for ki in range(2):
                        nc.tensor.matmul(
                            ps_pj[:, C * half:C * (half + 1)],
                            _r(oT_sb[ki][:, 128 * rb:128 * (rb + 1)]),
                            _r(wp_sb[ki][:]),
                            start=(ki == 0),
                            stop=(ki == 1 and not with_bp))
                    if with_bp:
                        nc.tensor.matmul(
                            ps_pj[:, C * half:C * (half + 1)],
                            _r(onesr_sb[0:1, :]),
                            _r(bp_sb[0:1, :]),
                            start=False, stop=True)
                st = sb_st.tile([128, BLK], F32, tag="st", name=f"st{b}{blk}{rbp}")
                nc.vector.tensor_copy(st[:], ps_pj[:])
                r0 = 128 * (4 * blk + 2 * rbp)
                dst = out[b, r0:r0 + 256, :].rearrange("(p r) c -> r p c", p=2)
                nc.sync.dma_start(dst, st.rearrange("r (p c) -> r p c", p=2))


def build(with_bp):
    nc = bacc.Bacc("TRN2", target_bir_lowering=False, debug=False,
                   enable_asserts=True)

    def din(name, shape, dtype=F32):
        return nc.dram_tensor(name, shape, dtype, kind="ExternalInput").ap()

    ins = [
        din("xT", [B_LOC, C, N], F32R),
        din("wq", [C, C], F32R),
        din("wk", [C, C], F32R),
        din("wv", [C, C], F32R),
        din("srw", [16, C, C], F32R),
        din("srb", [128, C]),
        din("bk", [2, 128, 1]),
        din("bv", [128, C]),
        din("wp", [C, C], F32R),
        din("bp", [1, C], F32R),
        din("eye", [128, 128]),
        din("ones32", [128, 32], BF16),
        din("onesr", [1, 128], F32R),
    ]
    out = nc.dram_tensor("out", [B_LOC, N, C], F32, kind="ExternalOutput").ap()

    with tile.TileContext(nc) as tc:
        with ExitStack() as ctx:
            _kernel_body(ctx, tc, out, ins, with_bp)
    nc.compile()
    return nc


def host_prep(inputs):
    """Shared (non-x) host-side tensors, from the full input dict."""
    Wq = np.asarray(inputs["Wq"], np.float32)
    Wkv = np.asarray(inputs["Wkv"], np.float32)
    sr_w = np.asarray(inputs["sr_w"], np.float32)
    sr_b = np.asarray(inputs["sr_b"], np.float32)
    ln_g = np.asarray(inputs["ln_g"], np.float32)
    ln_b = np.asarray(inputs["ln_b"], np.float32)
    Wp = np.asarray(inputs["Wp"], np.float32)
    bp = np.asarray(inputs["bp"], np.float32)

    wq = (Wq * (HD ** -0.5)).astype(np.float32)
    wk = (ln_g[:, None] * Wkv[:, :C]).astype(np.float32)
    wv = (ln_g[:, None] * Wkv[:, C:]).astype(np.float32)
    bias_kv = (ln_b @ Wkv).astype(np.float32)
    srwT = np.ascontiguousarray(
        sr_w.transpose(2, 3, 1, 0).reshape(16, C, C))

    shared = {
        "wq": wq,
        "wk": wk,
        "wv": wv,
        "srw": srwT,
        "srb": np.ascontiguousarray(np.broadcast_to(sr_b, (128, C))),
        "bk": np.ascontiguousarray(bias_kv[:C].reshape(2, 128, 1)),
        "bv": np.ascontiguousarray(np.broadcast_to(bias_kv[C:], (128, C))),
        "wp": Wp,
        "bp": np.ascontiguousarray(bp.reshape(1, C)),
        "eye": np.eye(128, dtype=np.float32),
        "ones32": np.ones((128, 32), ml_dtypes.bfloat16),
        "onesr": np.ones((1, 128), np.float32),
    }
    with_bp = bool(np.any(bp != 0))
    return shared, with_bp


_NC_CACHE = {}


def get_nc(with_bp):
    if with_bp not in _NC_CACHE:
        _NC_CACHE[with_bp] = build(with_bp)
    return _NC_CACHE[with_bp]


def _im2col_perm():
    """idx[tap*256 + m] = spatial row index n for the stride-4 4x4 conv."""
    tap = np.arange(16)
    kh, kw = tap // 4, tap % 4
    m = np.arange(256)
    R, Cc = m // 16, m % 16
    idx = (256 * R[None, :] + 4 * Cc[None, :]
           + 64 * kh[:, None] + kw[:, None])
    return idx.reshape(-1)


IM2COL_IDX = _im2col_perm()


def make_in_maps(inputs):
    x = np.asarray(inputs["x"], np.float32)
    shared, with_bp = host_prep(inputs)
    in_maps = []
    for c in range(N_CORES):
        xc = x[B_LOC * c:B_LOC * (c + 1)]
        xT = np.ascontiguousarray(xc.transpose(0, 2, 1)[:, :, IM2COL_IDX])
        m = dict(shared)
        m["xT"] = xT
        in_maps.append(m)
    return in_maps, with_bp


def kernel(**inputs):
    in_maps, with_bp = make_in_maps(inputs)
    nc = get_nc(with_bp)
    res = run_bass_kernel_spmd(nc, in_maps, core_ids=list(range(N_CORES)))
    KERNEL_STATS["exec_time_ns"] = res.exec_time_ns
    KERNEL_STATS["mean_exec_time_ns"] = res.mean_exec_time_ns
    KERNEL_STATS["trace"] = res.instructions_and_trace
    out_perm = np.concatenate(
        [res.results[c]["out"] for c in range(N_CORES)], axis=0)
    out = np.empty_like(out_perm)
    out[:, IM2COL_IDX, :] = out_perm
    return out


# revision 9
# speedup vs baseline: 1.3778x; 1.0862x over previous
"""PVT-style spatial-reduction attention on 8 TRN2 NeuronCores (Bass/Tile).

Strategy: data-parallel over batch (16 images -> 2 per core). Each core runs an
identical single-core program on its shard; no collectives.

Host-side prep (inside kernel(), part of sharding/layout):
  - x transposed to channel-major xT [2, 256, 4096] so matmul operands need no
    on-device transposition of the big activation.
  - attention scale hd^-0.5 folded into Wq; LayerNorm gamma/beta folded into
    Wkv algebraically (exact); conv weights pre-transposed per tap.

Device pipeline per batch (all matmuls float32r):
  qT = Wq^T @ xT                       (feature-major q)
  xr = sum over 16 conv taps of gathered-xT^T @ w_tap   (strided-gather lhsT)
  LN over free dim (quake rsqrt on DVE), PE-transpose of x_norm (small)
  kT = Wk^T @ xnT ; v = xn @ Wv
  per 512-row block, per head:
    S^T = kT_h^T @ qT_h  (keys on partitions)  -> exp on ScalarE (PSUM->SBUF)
    sums = ones32^T @ P^T  (32-replicated row sums via matmul)
    O^T  = v_h^T @ P^T     (unnormalized)
    R = reciprocal_approx_fast(sums); O-norm fused into PSUM->SBUF move
  out = O_norm^T^T @ Wp (+bp via K=1 matmul)  -> natural layout -> DMA out
"""

import os
import sys
from contextlib import ExitStack

if "/opt/trn_rl_repo" not in sys.path:
    sys.path.insert(0, "/opt/trn_rl_repo")

import numpy as np
import ml_dtypes

import concourse.bass as bass
import concourse.bacc as bacc
import concourse.tile as tile
from concourse import mybir
from concourse.bass_utils import run_bass_kernel_spmd

N_CORES = 8
B, N, C = 16, 4096, 256
B_LOC = B // N_CORES
H8, HD, M = 8, 32, 256
NBLK, BLK = 8, 512
F32 = mybir.dt.float32
F32R = mybir.dt.float32r
BF16 = mybir.dt.bfloat16
I32 = mybir.dt.int32
AF = mybir.ActivationFunctionType
OP = mybir.AluOpType
AX = mybir.AxisListType

KERNEL_STATS = {}


def _r(ap):
    return ap


def _kernel_body(ctx, tc, out, ins, with_bp):
    nc = tc.nc
    (xT_d, wq_d, wk_d, wv_d, srw_d, srb_d, bk_d, bv_d, wp_d, bp_d,
     eye_d, ones32_d, onesr_d) = ins

    consts = ctx.enter_context(tc.tile_pool(name="consts", bufs=1))
    sb_xT = ctx.enter_context(tc.tile_pool(name="sb_xT", bufs=2))
    sb_qT = ctx.enter_context(tc.tile_pool(name="sb_qT", bufs=1))
    sb_oT = ctx.enter_context(tc.tile_pool(name="sb_oT", bufs=1))
    sb_pT = ctx.enter_context(tc.tile_pool(name="sb_pT", bufs=8))
    sb_srw = ctx.enter_context(tc.tile_pool(name="sb_srw", bufs=8))
    sb_kv = ctx.enter_context(tc.tile_pool(name="sb_kv", bufs=2))
    sb_ln = ctx.enter_context(tc.tile_pool(name="sb_ln", bufs=2))
    sb_R = ctx.enter_context(tc.tile_pool(name="sb_R", bufs=2))
    sb_st = ctx.enter_context(tc.tile_pool(name="sb_st", bufs=3))
    ps_s = ctx.enter_context(tc.tile_pool(name="ps_s", bufs=2, space="PSUM"))
    ps_att = ctx.enter_context(tc.tile_pool(name="ps_att", bufs=2, space="PSUM"))
    ps_gen = ctx.enter_context(tc.tile_pool(name="ps_gen", bufs=2, space="PSUM"))

    # ---- constants / weights (resident) ----
    def cload(name, src, shape, dtype=F32):
        t = consts.tile(shape, dtype, tag=name, name=name)
        nc.sync.dma_start(t[:], src)
        return t

    wq_sb = [cload(f"wq{k}", wq_d[128 * k:128 * (k + 1), :], [128, C], F32R) for k in range(2)]
    wk_sb = [cload(f"wk{k}", wk_d[128 * k:128 * (k + 1), :], [128, C], F32R) for k in range(2)]
    wv_sb = [cload(f"wv{k}", wv_d[128 * k:128 * (k + 1), :], [128, C], F32R) for k in range(2)]
    wp_sb = [cload(f"wp{k}", wp_d[128 * k:128 * (k + 1), :], [128, C], F32R) for k in range(2)]
    srb_sb = cload("srb", srb_d[:, :], [128, C])
    bv_sb = cload("bv", bv_d[:, :], [128, C])
    bk_sb = [cload(f"bk{k}", bk_d[k], [128, 1]) for k in range(2)]
    eye_sb = cload("eye", eye_d[:, :], [128, 128])
    ones32_sb = cload("ones32", ones32_d[:, :], [128, 32], BF16)
    onesr_sb = cload("onesr", onesr_d[:, :], [1, 128], F32R)
    bp_sb = cload("bp", bp_d[:, :], [1, C], F32R)

    magic_t = consts.tile([128, 1], I32, tag="magic", name="magic")
    nc.gpsimd.memset(magic_t[:], 0x5F3759DF)
    c15_t = consts.tile([128, 1], F32, tag="c15", name="c15")
    nc.gpsimd.memset(c15_t[:], 1.5)

    for b in range(B_LOC):
        # ---- load xT ----
        xT_sb = []
        for k in range(2):
            xt = sb_xT.tile([128, N], F32R, tag="xT", name=f"xt{b}{k}")
            for q4 in range(4):
                nc.sync.dma_start(xt[:, 1024 * q4:1024 * (q4 + 1)],
                                  xT_d[b, 128 * k:128 * (k + 1),
                                       1024 * q4:1024 * (q4 + 1)])
            xT_sb.append(xt)

        # ---- conv (16 taps, accumulate in PSUM) ----
        # x arrives im2col-permuted: free index tap*256 + m, m = out position.
        ps_conv = [ps_gen.tile([128, C], F32, tag="g", name=f"psc{b}{mo}")
                   for mo in range(2)]
        for tap in range(16):
            srw_t = sb_srw.tile([128, 2 * C], F32R, tag="srw", name=f"srw{b}{tap}")
            nc.sync.dma_start(
                srw_t.rearrange("p (k o) -> p k o", k=2),
                srw_d[tap].rearrange("(k i) o -> i k o", k=2))
            for mo in range(2):
                for ki in range(2):
                    lhsT = xT_sb[ki][:, 256 * tap + 128 * mo:
                                     256 * tap + 128 * (mo + 1)]
                    nc.tensor.matmul(
                        ps_conv[mo][:],
                        _r(lhsT),
                        _r(srw_t[:, C * ki:C * (ki + 1)]),
                        start=(tap == 0 and ki == 0),
                        stop=(tap == 15 and ki == 1),
                    )

        # ---- LayerNorm (free-dim stats; gamma/beta folded into Wkv) ----
        xn_sb = []
        for mo in range(2):
            xb = sb_ln.tile([128, C], F32, tag="xb", name=f"xb{b}{mo}")
            nc.vector.tensor_add(xb[:], ps_conv[mo][:], srb_sb[:])
            ssum = sb_ln.tile([128, 1], F32, tag="ssum", name=f"ssum{b}{mo}")
            nc.vector.tensor_reduce(ssum[:], xb[:], axis=AX.X, op=OP.add)
            mu = sb_ln.tile([128, 1], F32, tag="mu", name=f"mu{b}{mo}")
            nc.vector.tensor_scalar_mul(mu[:], ssum[:], 1.0 / C)
            xc = sb_ln.tile([128, C], F32, tag="xc", name=f"xc{b}{mo}")
            nc.vector.tensor_scalar_sub(xc[:], xb[:], mu[:, 0:1])
            sq = sb_ln.tile([128, C], F32, tag="sq", name=f"sq{b}{mo}")
            vraw = sb_ln.tile([128, 1], F32, tag="vraw", name=f"vraw{b}{mo}")
            nc.vector.scalar_tensor_tensor(
                sq[:], xc[:], 0.0, xc[:], op0=OP.add, op1=OP.mult,
                accum_out=vraw[:, 0:1])
            veps = sb_ln.tile([128, 1], F32, tag="veps", name=f"veps{b}{mo}")
            nc.vector.tensor_scalar(veps[:], vraw[:], 1.0 / C, 1e-5,
                                    op0=OP.mult, op1=OP.add)
            vh = sb_ln.tile([128, 1], F32, tag="vh", name=f"vh{b}{mo}")
            nc.vector.tensor_scalar_mul(vh[:], veps[:], -0.5)
            # quake rsqrt seed + 3 Newton iterations
            sh = sb_ln.tile([128, 1], I32, tag="sh", name=f"sh{b}{mo}")
            nc.vector.tensor_scalar(sh[:], veps[:].bitcast(I32), 1, None,
                                    op0=OP.logical_shift_right)
            y = sb_ln.tile([128, 1], F32, tag="y", name=f"y{b}{mo}")
            nc.vector.scalar_tensor_tensor(
                y[:].bitcast(I32), magic_t[:], 0, sh[:],
                op0=OP.bypass, op1=OP.subtract)
            for it in range(3):
                yy = sb_ln.tile([128, 1], F32, tag=f"yy{it}", name=f"yy{b}{mo}{it}")
                nc.vector.tensor_mul(yy[:], y[:], y[:])
                t2 = sb_ln.tile([128, 1], F32, tag=f"t2{it}", name=f"t2{b}{mo}{it}")
                nc.vector.scalar_tensor_tensor(
                    t2[:], yy[:], vh[:, 0:1], c15_t[:],
                    op0=OP.mult, op1=OP.add)
                y2 = sb_ln.tile([128, 1], F32, tag=f"y2{it}", name=f"ynew{b}{mo}{it}")
                nc.vector.tensor_mul(y2[:], y[:], t2[:])
                y = y2
            xn = sb_ln.tile([128, C], F32, tag="xn", name=f"xn{b}{mo}")
            nc.vector.tensor_scalar_mul(xn[:], xc[:], y[:, 0:1])
            xn_sb.append(xn)

        # ---- transpose x_norm -> xnT [ci, pos] ----
        xnT_sb = []
        for i in range(2):
            xnT = sb_kv.tile([128, M], F32R, tag=f"xnT{i}", name=f"xnT{b}{i}")
            xnT_sb.append(xnT)
        for i in range(2):
            for j in range(2):
                ps_t = ps_gen.tile([128, 128], F32, tag="g", name=f"pst{b}{i}{j}")
                nc.tensor.transpose(ps_t[:], xn_sb[j][:, 128 * i:128 * (i + 1)],
                                    eye_sb[:])
                nc.vector.tensor_copy(xnT_sb[i][:, 128 * j:128 * (j + 1)], ps_t[:])

        # ---- kT = Wk^T @ xnT (+bias_k), v = xn @ Wv (+bias_v) ----
        kT_sb, v_sb = [], []
        for mo in range(2):
            ps_k = ps_gen.tile([128, M], F32, tag="g", name=f"psk{b}{mo}")
            for ki in range(2):
                nc.tensor.matmul(ps_k[:], _r(wk_sb[ki][:, 128 * mo:128 * (mo + 1)]),
                                 _r(xnT_sb[ki][:]),
                                 start=(ki == 0), stop=(ki == 1))
            kT = sb_kv.tile([128, M], BF16, tag=f"kT{mo}", name=f"kT{b}{mo}")
            nc.vector.tensor_scalar_add(kT[:], ps_k[:], bk_sb[mo][:, 0:1])
            kT_sb.append(kT)
        for mo in range(2):
            ps_v = ps_gen.tile([128, C], F32, tag="g", name=f"psv{b}{mo}")
            for ki in range(2):
                nc.tensor.matmul(ps_v[:], _r(xnT_sb[ki][:, 128 * mo:128 * (mo + 1)]),
                                 _r(wv_sb[ki][:]),
                                 start=(ki == 0), stop=(ki == 1))
            v = sb_kv.tile([128, C], BF16, tag=f"v{mo}", name=f"v{b}{mo}")
            nc.vector.tensor_add(v[:], ps_v[:], bv_sb[:])
            v_sb.append(v)

        # ---- qT = Wq^T @ xT ----
        qT_sb = [sb_qT.tile([128, N], BF16, tag=f"qT{k}", name=f"qT{b}{k}")
                 for k in range(2)]
        for blk in range(NBLK):
            for mo in range(2):
                ps_q = ps_gen.tile([128, BLK], F32, tag="g", name=f"psq{b}{blk}{mo}")
                for ki in range(2):
                    nc.tensor.matmul(
                        ps_q[:], _r(wq_sb[ki][:, 128 * mo:128 * (mo + 1)]),
                        _r(xT_sb[ki][:, BLK * blk:BLK * (blk + 1)]),
                        start=(ki == 0), stop=(ki == 1))
                nc.vector.tensor_copy(qT_sb[mo][:, BLK * blk:BLK * (blk + 1)],
                                      ps_q[:])

        # ---- attention blocks ----
        oT_sb = [sb_oT.tile([128, N], F32R, tag=f"oT{k}", name=f"oT{b}{k}")
                 for k in range(2)]
        for blk in range(NBLK):
            for sg in range(2):
                # S^T + exp, two heads at a time (adjacent row strips overlap
                # on the PE array); then 4-way col-packed sums and O matmuls.
                pts = []
                for pr in range(2):
                    st_ts = []
                    for hp in range(2):
                        hl = 2 * pr + hp
                        hh = 4 * sg + hl
                        st_t = ps_s.tile([128, 2 * BLK], F32, tag="s",
                                         name=f"psst{b}{blk}{hh}")
                        st_ts.append(st_t)
                    for ko in range(2):
                        for hp in range(2):
                            hl = 2 * pr + hp
                            nc.tensor.matmul(
                                st_ts[hp][:, BLK * ko:BLK * (ko + 1)],
                                _r(kT_sb[sg][32 * hl:32 * hl + 32,
                                             128 * ko:128 * (ko + 1)]),
                                _r(qT_sb[sg][32 * hl:32 * hl + 32,
                                             BLK * blk:BLK * (blk + 1)]),
                                start=True, stop=True,
                                tile_position=(32 * hl, 0),
                            )
                    for hp in range(2):
                        hh = 4 * sg + 2 * pr + hp
                        pt = sb_pT.tile([128, 2 * BLK], BF16, tag="pT",
                                        name=f"pt{b}{blk}{hh}")
                        nc.scalar.activation(pt[:], st_ts[hp][:], AF.Exp)
                        pts.append(pt)
                ps_sum = ps_att.tile([128, BLK], F32, tag="att",
                                     name=f"pssum{b}{blk}{sg}")
                ps_o = ps_att.tile([128, BLK], F32, tag="att",
                                   name=f"pso{b}{blk}{sg}")
                for ko in range(2):
                    for hl in range(4):
                        nc.tensor.matmul(
                            ps_sum[32 * hl:32 * hl + 32, :],
                            _r(ones32_sb[:]),
                            _r(pts[hl][:, BLK * ko:BLK * (ko + 1)]),
                            start=(ko == 0), stop=(ko == 1),
                            tile_position=(0, 32 * hl),
                            skip_group_check=True,
                        )
                for ko in range(2):
                    for hl in range(4):
                        hh = 4 * sg + hl
                        nc.tensor.matmul(
                            ps_o[32 * hl:32 * hl + 32, :],
                            _r(v_sb[ko][:, 32 * hh:32 * hh + 32]),
                            _r(pts[hl][:, BLK * ko:BLK * (ko + 1)]),
                            start=(ko == 0), stop=(ko == 1),
                            tile_position=(0, 32 * hl),
                            skip_group_check=True,
                        )
                R_t = sb_R.tile([128, BLK], F32, tag="R", name=f"R{b}{blk}{sg}")
                nc.vector.reciprocal_approx_fast(R_t[:], ps_sum[:])
                nc.vector.tensor_mul(oT_sb[sg][:, BLK * blk:BLK * (blk + 1)],
                                     ps_o[:], R_t[:])

            # ---- proj for this block's 4 row-chunks ----
            for rbp in range(2):
                ps_pj = ps_gen.tile([128, BLK], F32, tag="g",
                                    name=f"pspj{b}{blk}{rbp}")
                for half in range(2):
                    rb = 4 * blk + 2 * rbp + half
                    for ki in range(2):
                        nc.tensor.matmul(
                            ps_pj[:, C * half:C * (half + 1)],
                            _r(oT_sb[ki][:, 128 * rb:128 * (rb + 1)]),
                            _r(wp_sb[ki][:]),
                            start=(ki == 0),
                            stop=(ki == 1 and not with_bp))
                    if with_bp:
                        nc.tensor.matmul(
                            ps_pj[:, C * half:C * (half + 1)],
                            _r(onesr_sb[0:1, :]),
                            _r(bp_sb[0:1, :]),
                            start=False, stop=True)
                st = sb_st.tile([128, BLK], F32, tag="st", name=f"st{b}{blk}{rbp}")
                nc.vector.tensor_copy(st[:], ps_pj[:])
                r0 = 128 * (4 * blk + 2 * rbp)
                dst = out[b, r0:r0 + 256, :].rearrange("(p r) c -> r p c", p=2)
                nc.sync.dma_start(dst, st.rearrange("r (p c) -> r p c", p=2))


def build(with_bp):
    nc = bacc.Bacc("TRN2", target_bir_lowering=False, debug=False,
                   enable_asserts=True)

    def din(name, shape, dtype=F32):
        return nc.dram_tensor(name, shape, dtype, kind="ExternalInput").ap()

    ins = [
        din("xT", [B_LOC, C, N], F32R),
        din("wq", [C, C], F32R),
        din("wk", [C, C], F32R),
        din("wv", [C, C], F32R),
        din("srw", [16, C, C], F32R),
        din("srb", [128, C]),
        din("bk", [2, 128, 1]),
        din("bv", [128, C]),
        din("wp", [C, C], F32R),
        din("bp", [1, C], F32R),
        din("eye", [128, 128]),
        din("ones32", [128, 32], BF16),
        din("onesr", [1, 128], F32R),
    ]
    out = nc.dram_tensor("out", [B_LOC, N, C], F32, kind="ExternalOutput").ap()

    with tile.TileContext(nc) as tc:
        with ExitStack() as ctx:
            _kernel_body(ctx, tc, out, ins, with_bp)
    nc.compile()
    return nc


def host_prep(inputs):
    """Shared (non-x) host-side tensors, from the full input dict."""
    Wq = np.asarray(inputs["Wq"], np.float32)
    Wkv = np.asarray(inputs["Wkv"], np.float32)
    sr_w = np.asarray(inputs["sr_w"], np.float32)
    sr_b = np.asarray(inputs["sr_b"], np.float32)
    ln_g = np.asarray(inputs["ln_g"], np.float32)
    ln_b = np.asarray(inputs["ln_b"], np.float32)
    Wp = np.asarray(inputs["Wp"], np.float32)
    bp = np.asarray(inputs["bp"], np.float32)

    wq = (Wq * (HD ** -0.5)).astype(np.float32)
    wk = (ln_g[:, None] * Wkv[:, :C]).astype(np.float32)
    wv = (ln_g[:, None] * Wkv[:, C:]).astype(np.float32)
    bias_kv = (ln_b @ Wkv).astype(np.float32)
    srwT = np.ascontiguousarray(
        sr_w.transpose(2, 3, 1, 0).reshape(16, C, C))

    shared = {
        "wq": wq,
        "wk": wk,
        "wv": wv,
        "srw": srwT,
        "srb": np.ascontiguousarray(np.broadcast_to(sr_b, (128, C))),
        "bk": np.ascontiguousarray(bias_kv[:C].reshape(2, 128, 1)),
        "bv": np.ascontiguousarray(np.broadcast_to(bias_kv[C:], (128, C))),
        "wp": Wp,
        "bp": np.ascontiguousarray(bp.reshape(1, C)),
        "eye": np.eye(128, dtype=np.float32),
        "ones32": np.ones((128, 32), ml_dtypes.bfloat16),
        "onesr": np.ones((1, 128), np.float32),
    }
    with_bp = bool(np.any(bp != 0))
    return shared, with_bp


_NC_CACHE = {}


def get_nc(with_bp):
    if with_bp not in _NC_CACHE:
        _NC_CACHE[with_bp] = build(with_bp)
    return _NC_CACHE[with_bp]


def _im2col_perm():
    """idx[tap*256 + m] = spatial row index n for the stride-4 4x4 conv."""
    tap = np.arange(16)
    kh, kw = tap // 4, tap % 4
    m = np.arange(256)
    R, Cc = m // 16, m % 16
    idx = (256 * R[None, :] + 4 * Cc[None, :]
           + 64 * kh[:, None] + kw[:, None])
    return idx.reshape(-1)


IM2COL_IDX = _im2col_perm()


def make_in_maps(inputs):
    x = np.asarray(inputs["x"], np.float32)
    shared, with_bp = host_prep(inputs)
    in_maps = []
    for c in range(N_CORES):
        xc = x[B_LOC * c:B_LOC * (c + 1)]
        xT = np.ascontiguousarray(xc.transpose(0, 2, 1)[:, :, IM2COL_IDX])
        m = dict(shared)
        m["xT"] = xT
        in_maps.append(m)
    return in_maps, with_bp


def kernel(**inputs):
    in_maps, with_bp = make_in_maps(inputs)
    nc = get_nc(with_bp)
    res = run_bass_kernel_spmd(nc, in_maps, core_ids=list(range(N_CORES)))
    KERNEL_STATS["exec_time_ns"] = res.exec_time_ns
    KERNEL_STATS["mean_exec_time_ns"] = res.mean_exec_time_ns
    KERNEL_STATS["trace"] = res.instructions_and_trace
    out_perm = np.concatenate(
        [res.results[c]["out"] for c in range(N_CORES)], axis=0)
    out = np.empty_like(out_perm)
    out[:, IM2COL_IDX, :] = out_perm
    return out
